# revision 1
# baseline (speedup 1.0000x reference)
"""CrossAttention (DFFNet) Trainium2 Bass kernel.

Shapes (hardcoded): rgb/depth [4, 256, 64, 64] f32; Wq/Wk [32, 256]; Wv [256, 256].

    q = Wq @ d + bq          [B, 32, 4096]
    k = Wk @ d + bk          [B, 32, 4096]
    v = Wv @ r + bv          [B, 256, 4096]
    scores = q^T k           [B, 4096, 4096], softmax over keys (last dim)
    feat = v @ mask^T        [B, 256, 4096]

Sharding: 8 cores = 4 batches x 2 query-halves (2048 queries each). Each core
gets full depth/rgb for its batch (keys/values span all 4096 tokens) plus its
query-half slice of depth.

Device layout choice: scores are computed TRANSPOSED, st[m, n] (keys m on
partitions, queries n free) so the feat matmul needs no transposes:
  - v^T[m, c] is produced directly by  r-slice^T @ Wv^T  (r already has
    channels on partitions, which is the contraction dim).
  - feat[c, n] = sum_m v^T[m, c] * exp(st[m, n]) / S[n]  -> lhsT = v^T tile,
    rhs = exp(st) tile, both with m on partitions.
  - softmax denominator S[n] = sum_m exp(st[m,n]) is a partition-axis sum ->
    ones[128,1]-lhsT matmul accumulated over m-tiles (PE streams it).
  - no max-subtraction: |scores| < ~6 here, exp is well-conditioned.
Normalization: 1/S via fast reciprocal (DVE), broadcast to 128 partitions via
a K=1 matmul with a ones row, multiply + bias-add on DVE.

The K=32 score matmuls are 4-way row-packed (tile_position=(32j, 0)): k and q
are kept in 4x-replicated layouts [128, *] (4 copies at partition offsets
0/32/64/96), which fall out of the projection matmuls for free by tiling the
tiny weight matrices host-side (WkT_4x = tile(Wk.T, (1, 4))).

All matmuls run as float32r (full PE rate at free-dim >= 256, fp32 storage).
"""

import numpy as np

import concourse.bacc as bacc
import concourse.bass as bass
import concourse.mybir as mybir
import concourse.tile as tile
from concourse.bass_utils import run_bass_kernel_spmd

B, C, H, W = 4, 256, 64, 64
HW = H * W            # 4096
CQK = 32
P = 128
NQ = HW // 2          # 2048 queries per core
NT = 512              # query tile
N_NT = NQ // NT       # 4
MT = HW // P          # 32 key tiles
KC = C // P           # 2 contraction tiles for the projections

F32 = mybir.dt.float32
F32R = mybir.dt.float32r
BF16 = mybir.dt.bfloat16
AF = mybir.ActivationFunctionType
OP = mybir.AluOpType


def _r(ap):
    """View an fp32 AP as float32r (valid only after _round_inplace)."""
    return ap.bitcast(F32R)


def _staged_load(nc, pool, dst, dram_ap, chunk=1024):
    """DMA fp32 DRAM -> small staging tile, DVE-copy (rounding) -> f32r dst."""
    n = dst.shape[1]
    for c0 in range(0, n, chunk):
        w = min(chunk, n - c0)
        stg = pool.tile([P, chunk], F32, tag="stage", name=f"stg_{dst.name}_{c0}")
        nc.sync.dma_start(stg[:, 0:w], dram_ap[:, c0:c0 + w])
        nc.vector.tensor_copy(dst[:, c0:c0 + w], stg[:, 0:w])


def _emit(tc, io):
    nc = tc.nc
    d = io["d"].ap()          # [256, 4096] depth (keys source)
    dq = io["dq"].ap()        # [256, 2048] depth query-half
    r = io["r"].ap()          # [256, 4096] rgb (values source)
    wqt4 = io["wqt4"].ap()    # [256, 128] = tile(Wq.T, (1,4))
    wkt4 = io["wkt4"].ap()    # [256, 128]
    wvt = io["wvt"].ap()      # [256, 256] = Wv.T
    bq4 = io["bq4"].ap()      # [128, 1] = tile(bq, 4)
    bk4 = io["bk4"].ap()      # [128, 1]
    bv2 = io["bv2"].ap()      # [256, 1]
    out = io["out"].ap()      # [256, 2048]

    from contextlib import ExitStack

    with ExitStack() as ctx:
        pw = ctx.enter_context(tc.tile_pool(name="weights", bufs=1))
        pin = ctx.enter_context(tc.tile_pool(name="inputs", bufs=1))
        pqk = ctx.enter_context(tc.tile_pool(name="qk", bufs=1))
        pvt = ctx.enter_context(tc.tile_pool(name="vt", bufs=1))
        pse = ctx.enter_context(tc.tile_pool(name="stexp", bufs=2))
        psmall = ctx.enter_context(tc.tile_pool(name="small", bufs=2))
        pout = ctx.enter_context(tc.tile_pool(name="outsb", bufs=4))
        pstage = ctx.enter_context(tc.tile_pool(name="stage", bufs=4))
        ps_st = ctx.enter_context(
            tc.tile_pool(name="ps_st", bufs=2, space=bass.MemorySpace.PSUM))
        ps_feat = ctx.enter_context(
            tc.tile_pool(name="ps_feat", bufs=2, space=bass.MemorySpace.PSUM))
        ps_sums = ctx.enter_context(
            tc.tile_pool(name="ps_sums", bufs=1, space=bass.MemorySpace.PSUM))
        ps_bc = ctx.enter_context(
            tc.tile_pool(name="ps_bc", bufs=1, space=bass.MemorySpace.PSUM))

        # ---- constants / weights --------------------------------------
        wq_t, wk_t, wv_t, bv_t = [], [], [], []
        for kc in range(KC):
            t = pw.tile([P, P], BF16, tag=f"wq{kc}")
            _staged_load(nc, pstage, t, wqt4[kc * P:(kc + 1) * P, :])
            wq_t.append(t)
            t = pw.tile([P, P], BF16, tag=f"wk{kc}")
            _staged_load(nc, pstage, t, wkt4[kc * P:(kc + 1) * P, :])
            wk_t.append(t)
            t = pw.tile([P, C], BF16, tag=f"wv{kc}")
            _staged_load(nc, pstage, t, wvt[kc * P:(kc + 1) * P, :])
            wv_t.append(t)
            t = pw.tile([P, 1], F32, tag=f"bv{kc}")
            nc.sync.dma_start(t[:], bv2[kc * P:(kc + 1) * P, :])
            bv_t.append(t)
        bq_sb = pw.tile([P, 1], F32, tag="bq")
        nc.sync.dma_start(bq_sb[:], bq4[:])
        bk_sb = pw.tile([P, 1], F32, tag="bk")
        nc.sync.dma_start(bk_sb[:], bk4[:])
        ones_f = pw.tile([P, 1], F32, tag="ones_f")
        nc.vector.memset(ones_f[:], 1.0)
        ones_col = pw.tile([P, 1], BF16, tag="ones_col")
        nc.vector.tensor_copy(ones_col[:], ones_f[:])
        ones_row = pw.tile([1, P], F32, tag="ones_row")
        nc.vector.memset(ones_row[:], 1.0)

        # ---- inputs (per-chunk tiles so projections start early) -------
        def _chunked(dram_ap, kc, nch, pref):
            tiles = []
            for ch in range(nch):
                t = pin.tile([P, 1024], BF16, tag=f"{pref}{kc}_{ch}",
                             name=f"{pref}{kc}_{ch}")
                stg = pstage.tile([P, 1024], F32, tag="stage",
                                  name=f"stg_{pref}{kc}_{ch}")
                nc.sync.dma_start(
                    stg[:], dram_ap[kc * P:(kc + 1) * P,
                                    ch * 1024:(ch + 1) * 1024])
                nc.vector.tensor_copy(t[:], stg[:])
                tiles.append(t)
            return tiles

        d_sb = [_chunked(d, kc, 4, "d") for kc in range(KC)]
        dq_sb = [_chunked(dq, kc, 2, "dq") for kc in range(KC)]
        r_sb = [_chunked(r, kc, 4, "r") for kc in range(KC)]

        # ---- k / q projections (4x-replicated layouts) -----------------
        # k4[32j + o, m] = k[o, m];  q4[32j + o, n] = q[o, n]
        k4 = pqk.tile([P, HW], BF16, tag="k4")
        for qtr in range(4):
            kp = ps_st.tile([P, 1024], F32, tag="stp", name=f"kp{qtr}")
            for sub in range(2):
                n0 = sub * NT
                g0 = qtr * 1024 + n0
                for kc in range(KC):
                    nc.tensor.matmul(
                        kp[:, n0:n0 + NT],
                        lhsT=wk_t[kc][:],
                        rhs=d_sb[kc][qtr][:, n0:n0 + NT],
                        start=(kc == 0),
                        stop=(kc == KC - 1),
                    )
            nc.vector.tensor_scalar(
                k4[:, qtr * 1024:(qtr + 1) * 1024], kp[:], bk_sb[:], None, OP.add
            )
        q4 = pqk.tile([P, NQ], BF16, tag="q4")
        for half in range(2):
            qp = ps_st.tile([P, 1024], F32, tag="stp", name=f"qp{half}")
            for sub in range(2):
                n0 = sub * NT
                g0 = half * 1024 + n0
                for kc in range(KC):
                    nc.tensor.matmul(
                        qp[:, n0:n0 + NT],
                        lhsT=wq_t[kc][:],
                        rhs=dq_sb[kc][half][:, n0:n0 + NT],
                        start=(kc == 0),
                        stop=(kc == KC - 1),
                    )
            nc.vector.tensor_scalar(
                q4[:, half * 1024:(half + 1) * 1024], qp[:], bq_sb[:], None, OP.add
            )

        # ---- v^T projection: vt[mt][p, c] = v[c, mt*128 + p] (no bias) --
        vt_t = []
        for mt in range(MT):
            vp = ps_feat.tile([P, C], F32, tag="feat")
            for kc in range(KC):
                nc.tensor.matmul(
                    vp[:],
                    lhsT=r_sb[kc][mt // 8][:, (mt % 8) * P:(mt % 8 + 1) * P],
                    rhs=wv_t[kc][:],
                    start=(kc == 0),
                    stop=(kc == KC - 1),
                )
            t = pvt.tile([P, C], BF16, tag=f"vt{mt}")
            nc.vector.tensor_copy(t[:], vp[:])
            vt_t.append(t)

        # ---- main attention loop ---------------------------------------
        for nt in range(N_NT):
            n0 = nt * NT
            fc = [ps_feat.tile([P, NT], F32, tag="feat", name=f"fc{nt}_{i}") for i in range(2)]
            sm = ps_sums.tile([1, NT], F32, tag="sums")
            for g in range(MT // 2):
                stp = ps_st.tile([P, 1024], F32, tag="stp", name=f"stp{nt}_{g}")
                for j in range(2):
                    mt = 2 * g + j
                    nc.tensor.matmul(
                        stp[:, j * NT:(j + 1) * NT],
                        lhsT=k4[32 * j:32 * j + 32, mt * P:(mt + 1) * P],
                        rhs=q4[32 * j:32 * j + 32, n0:n0 + NT],
                        start=True,
                        stop=True,
                        tile_position=(32 * j, 0),
                    )
                se = pse.tile([P, 1024], BF16, tag="se", name=f"se{nt}_{g}")
                nc.scalar.activation(se[:], stp[:], AF.Exp)
                for j in range(2):
                    mt = 2 * g + j
                    sej = se[:, j * NT:(j + 1) * NT]
                    first = mt == 0
                    last = mt == MT - 1
                    nc.tensor.matmul(
                        fc[0][:], lhsT=vt_t[mt][:, 0:P], rhs=sej,
                        start=first, stop=last,
                    )
                    nc.tensor.matmul(
                        fc[1][:], lhsT=vt_t[mt][:, P:C], rhs=sej,
                        start=first, stop=last,
                    )
                    nc.tensor.matmul(
                        sm[:], lhsT=ones_col[:], rhs=sej,
                        start=first, stop=last,
                    )
            rc = psmall.tile([1, NT], F32, tag="recip")
            nc.vector.reciprocal_approx_fast(out=rc[:], in_=sm[:])
            bc = ps_bc.tile([P, NT], F32, tag="bc")
            nc.tensor.matmul(
                bc[:], lhsT=ones_row[:], rhs=rc[:], start=True, stop=True
            )
            bc_sb = pout.tile([P, NT], F32, tag="bc_sb")
            nc.vector.tensor_copy(bc_sb[:], bc[:])
            for c in range(2):
                tmp = pout.tile([P, NT], F32, tag="tmp")
                nc.vector.tensor_tensor(tmp[:], fc[c][:], bc_sb[:], OP.mult)
                ot = pout.tile([P, NT], F32, tag="ot")
                nc.vector.tensor_scalar(ot[:], tmp[:], bv_t[c][:], None, OP.add)
                nc.sync.dma_start(out[c * P:(c + 1) * P, n0:n0 + NT], ot[:])


_BUILT = None


def _build():
    global _BUILT
    if _BUILT is not None:
        return _BUILT
    nc = bacc.Bacc("TRN2", target_bir_lowering=False, debug=False)
    io = {
        "d": nc.dram_tensor("d", [C, HW], F32, kind="ExternalInput"),
        "dq": nc.dram_tensor("dq", [C, NQ], F32, kind="ExternalInput"),
        "r": nc.dram_tensor("r", [C, HW], F32, kind="ExternalInput"),
        "wqt4": nc.dram_tensor("wqt4", [C, P], F32, kind="ExternalInput"),
        "wkt4": nc.dram_tensor("wkt4", [C, P], F32, kind="ExternalInput"),
        "wvt": nc.dram_tensor("wvt", [C, C], F32, kind="ExternalInput"),
        "bq4": nc.dram_tensor("bq4", [P, 1], F32, kind="ExternalInput"),
        "bk4": nc.dram_tensor("bk4", [P, 1], F32, kind="ExternalInput"),
        "bv2": nc.dram_tensor("bv2", [C, 1], F32, kind="ExternalInput"),
        "out": nc.dram_tensor("out", [C, NQ], F32, kind="ExternalOutput"),
    }
    with tile.TileContext(nc) as tc:
        _emit(tc, io)
    nc.compile()
    _BUILT = nc
    return nc


def _in_maps(rgb, depth, Wq, bq, Wk, bk, Wv, bv):
    f = np.float32
    d_all = np.ascontiguousarray(depth.reshape(B, C, HW), dtype=f)
    r_all = np.ascontiguousarray(rgb.reshape(B, C, HW), dtype=f)
    wqt4 = np.ascontiguousarray(np.tile(np.asarray(Wq, f).T, (1, 4)))
    wkt4 = np.ascontiguousarray(np.tile(np.asarray(Wk, f).T, (1, 4)))
    wvt = np.ascontiguousarray(np.asarray(Wv, f).T)
    bq4 = np.ascontiguousarray(np.tile(np.asarray(bq, f), 4).reshape(P, 1))
    bk4 = np.ascontiguousarray(np.tile(np.asarray(bk, f), 4).reshape(P, 1))
    bv2 = np.ascontiguousarray(np.asarray(bv, f).reshape(C, 1))
    maps = []
    for core in range(8):
        b, half = core // 2, core % 2
        maps.append({
            "d": d_all[b],
            "dq": np.ascontiguousarray(d_all[b][:, half * NQ:(half + 1) * NQ]),
            "r": r_all[b],
            "wqt4": wqt4, "wkt4": wkt4, "wvt": wvt,
            "bq4": bq4, "bk4": bk4, "bv2": bv2,
        })
    return maps


def kernel(rgb, depth, Wq, bq, Wk, bk, Wv, bv, **run_kwargs):
    nc = _build()
    maps = _in_maps(rgb, depth, Wq, bq, Wk, bk, Wv, bv)
    res = run_bass_kernel_spmd(nc, maps, core_ids=list(range(8)), **run_kwargs)
    results = res.results if hasattr(res, "results") else res
    out = np.empty((B, C, HW), dtype=np.float32)
    for core in range(8):
        b, half = core // 2, core % 2
        out[b][:, half * NQ:(half + 1) * NQ] = results[core]["out"]
    kernel.last_results = res
    return out.reshape(B, C, H, W)



# revision 13
# speedup vs baseline: 1.6131x; 1.6131x over previous
"""CrossAttention (DFFNet) Trainium2 Bass kernel.

Shapes (hardcoded): rgb/depth [4, 256, 64, 64] f32; Wq/Wk [32, 256]; Wv [256, 256].

    q = Wq @ d + bq          [B, 32, 4096]
    k = Wk @ d + bk          [B, 32, 4096]
    v = Wv @ r + bv          [B, 256, 4096]
    scores = q^T k           [B, 4096, 4096], softmax over keys (last dim)
    feat = v @ mask^T        [B, 256, 4096]

Sharding: 8 cores = 4 batches x 2 query-halves (2048 queries each). Each core
gets full depth/rgb for its batch (keys/values span all 4096 tokens) plus its
query-half slice of depth.

Device layout: scores are computed TRANSPOSED, st[m, n] (keys m on partitions,
queries n free) so the feat matmul needs no transposes:
  - v^T[m, c] is produced directly by  r-slice^T @ Wv^T.
  - feat[c, n] = sum_m v^T[m, c] * exp(st[m, n]) / S[n].
  - softmax denominator S[n] = sum_m exp(st[m,n]) via ones-lhsT matmul.
The PE array is the bottleneck (99% busy), so the exp tiles and v^T tiles are
stored as fp8-e4m3 and the feat + sums matmuls run in DoubleRow perf mode
(two 128-deep key tiles contracted per instruction at 0.5 cycles/row), halving
their PE streaming time. exp is computed with a built-in bias of -ln(8)
(out = exp(st)/8) to center values in fp8e4's range [2^-9, 240]; the scale
cancels exactly in the softmax normalization since the denominator uses the
same scaled tiles. Scores stay bf16 (K=32, output-bound, fp8 wouldn't help).

Inputs arrive from the host pre-cast to bf16 (halves DMA bytes and removes
all on-chip f32->bf16 staging casts). Normalization: 1/S via fast reciprocal
(f32), cast to fp16, broadcast to 128 partitions via a K=1 fp16 matmul.
"""

import math

import numpy as np
import ml_dtypes

import concourse.bacc as bacc
import concourse.bass as bass
import concourse.mybir as mybir
import concourse.tile as tile
from concourse.bass_utils import run_bass_kernel_spmd

B, C, H, W = 4, 256, 64, 64
HW = H * W            # 4096
CQK = 32
P = 128
NQ = HW // 2          # 2048 queries per core
NT = 512              # query tile
N_NT = NQ // NT       # 4
MT = HW // P          # 32 key tiles
NPAIR = MT // 2       # 16 DoubleRow key-tile pairs
KC = C // P           # 2 contraction tiles for the projections

F32 = mybir.dt.float32
F16 = mybir.dt.float16
BF16 = mybir.dt.bfloat16
FP8 = mybir.dt.float8e4
AF = mybir.ActivationFunctionType
OP = mybir.AluOpType
DR = mybir.MatmulPerfMode.DoubleRow

EXP_BIAS = -math.log(8.0)   # exp(st)/8: keeps weights in fp8e4 normal range


def _emit(tc, io):
    nc = tc.nc
    d = io["d"].ap()          # [256, 4096] bf16 depth (keys source)
    dq = io["dq"].ap()        # [256, 2048] bf16 depth query-half
    r = io["r"].ap()          # [256, 4096] bf16 rgb (values source)
    wqt4 = io["wqt4"].ap()    # [256, 128] bf16 = tile(Wq.T, (1,4))
    wkt4 = io["wkt4"].ap()    # [256, 128] bf16
    wvt = io["wvt"].ap()      # [256, 256] bf16 = Wv.T
    bq4 = io["bq4"].ap()      # [128, 1] f32 = tile(bq, 4)
    bk4 = io["bk4"].ap()      # [128, 1] f32
    bv2 = io["bv2"].ap()      # [256, 1] f32
    out = io["out"].ap()      # [256, 2048] f32

    from contextlib import ExitStack

    with ExitStack() as ctx:
        pw = ctx.enter_context(tc.tile_pool(name="weights", bufs=1))
        pin = ctx.enter_context(tc.tile_pool(name="inputs", bufs=1))
        pqk = ctx.enter_context(tc.tile_pool(name="qk", bufs=1))
        pvt = ctx.enter_context(tc.tile_pool(name="vt", bufs=1))
        pse = ctx.enter_context(tc.tile_pool(name="stexp", bufs=2))
        psmall = ctx.enter_context(tc.tile_pool(name="small", bufs=2))
        pout = ctx.enter_context(tc.tile_pool(name="outsb", bufs=4))
        ps_st = ctx.enter_context(
            tc.tile_pool(name="ps_st", bufs=2, space=bass.MemorySpace.PSUM))
        ps_feat = ctx.enter_context(
            tc.tile_pool(name="ps_feat", bufs=2, space=bass.MemorySpace.PSUM))
        ps_sums = ctx.enter_context(
            tc.tile_pool(name="ps_sums", bufs=1, space=bass.MemorySpace.PSUM))
        ps_vp = ctx.enter_context(
            tc.tile_pool(name="ps_vp", bufs=1, space=bass.MemorySpace.PSUM))

        # ---- constants / weights (inputs already bf16: direct DMA) -----
        wq_t, wk_t, wv_t, bv_t = [], [], [], []
        for kc in range(KC):
            t = pw.tile([P, P], BF16, tag=f"wq{kc}")
            nc.sync.dma_start(t[:], wqt4[kc * P:(kc + 1) * P, :])
            wq_t.append(t)
            t = pw.tile([P, P], BF16, tag=f"wk{kc}")
            nc.sync.dma_start(t[:], wkt4[kc * P:(kc + 1) * P, :])
            wk_t.append(t)
            t = pw.tile([P, C], BF16, tag=f"wv{kc}")
            nc.sync.dma_start(t[:], wvt[kc * P:(kc + 1) * P, :])
            wv_t.append(t)
            t = pw.tile([P, 1], F32, tag=f"bv{kc}")
            nc.sync.dma_start(t[:], bv2[kc * P:(kc + 1) * P, :])
            bv_t.append(t)
        bq_sb = pw.tile([P, 1], F32, tag="bq")
        nc.sync.dma_start(bq_sb[:], bq4[:])
        bk_sb = pw.tile([P, 1], F32, tag="bk")
        nc.sync.dma_start(bk_sb[:], bk4[:])
        # Full-width ones lhsT: dual-fp8 LDWEIGHTS requires col_grp=0xf (all
        # 128 PE columns), so a [128,2,1] ones vector is illegal. The [128,2,128]
        # form also lands S[n] pre-broadcast on all 128 PSUM partitions, which
        # removes the separate K=1 broadcast matmul from the critical path.
        ones2 = pw.tile([P, 2, P], FP8, tag="ones2")
        nc.vector.memset(ones2[:], 1.0)
        ebias = pw.tile([P, 1], F32, tag="ebias")
        nc.vector.memset(ebias[:], EXP_BIAS)

        # ---- inputs (per-chunk tiles so projections start early) -------
        # Spread the input DMAs over four engines' descriptor queues so the
        # 5MB load phase runs on multiple rings concurrently instead of
        # serializing behind nc.sync's queue.
        def _chunked(dram_ap, kc, nch, pref, eng):
            tiles = []
            for ch in range(nch):
                t = pin.tile([P, 1024], BF16, tag=f"{pref}{kc}_{ch}",
                             name=f"{pref}{kc}_{ch}")
                eng.dma_start(
                    t[:], dram_ap[kc * P:(kc + 1) * P,
                                  ch * 1024:(ch + 1) * 1024])
                tiles.append(t)
            return tiles

        d_sb = [_chunked(d, kc, 4, "d", [nc.scalar, nc.gpsimd][kc])
                for kc in range(KC)]
        dq_sb = [_chunked(dq, kc, 2, "dq", nc.sync) for kc in range(KC)]
        r_sb = [_chunked(r, kc, 4, "r", [nc.scalar, nc.gpsimd][kc])
                for kc in range(KC)]

        # ---- k / q projections (4x-replicated layouts) -----------------
        # k4[32j + o, m] = k[o, m];  q4[32j + o, n] = q[o, n]
        k4 = pqk.tile([P, HW], BF16, tag="k4")
        for qtr in range(4):
            kp = ps_st.tile([P, 2, NT], F32, tag="stp", name=f"kp{qtr}")
            for sub in range(2):
                for kc in range(KC):
                    nc.tensor.matmul(
                        kp[:, sub:sub + 1, :],
                        lhsT=wk_t[kc][:],
                        rhs=d_sb[kc][qtr][:, sub * NT:(sub + 1) * NT],
                        start=(kc == 0),
                        stop=(kc == KC - 1),
                    )
                nc.vector.tensor_scalar(
                    k4[:, qtr * 1024 + sub * NT:qtr * 1024 + (sub + 1) * NT],
                    kp[:, sub:sub + 1, :], bk_sb[:], None, OP.add,
                )
        q4 = pqk.tile([P, NQ], BF16, tag="q4")
        for half in range(2):
            qp = ps_st.tile([P, 2, NT], F32, tag="stp", name=f"qp{half}")
            for sub in range(2):
                for kc in range(KC):
                    nc.tensor.matmul(
                        qp[:, sub:sub + 1, :],
                        lhsT=wq_t[kc][:],
                        rhs=dq_sb[kc][half][:, sub * NT:(sub + 1) * NT],
                        start=(kc == 0),
                        stop=(kc == KC - 1),
                    )
                nc.vector.tensor_scalar(
                    q4[:, half * 1024 + sub * NT:half * 1024 + (sub + 1) * NT],
                    qp[:, sub:sub + 1, :], bq_sb[:], None, OP.add,
                )

        # ---- v^T projection: vtp[g][p, i, c] = v[c, (2g+i)*128 + p] ----
        # Stored fp8 in DoubleRow pair layout (no bias; bias added at the end).
        vtp = []
        for g in range(NPAIR):
            t = pvt.tile([P, 2, C], FP8, tag=f"vt{g}")
            vtp.append(t)
        for mt in range(MT):
            vp = ps_vp.tile([P, C], F32, tag="vp")
            for kc in range(KC):
                nc.tensor.matmul(
                    vp[:],
                    lhsT=r_sb[kc][mt // 8][:, (mt % 8) * P:(mt % 8 + 1) * P],
                    rhs=wv_t[kc][:],
                    start=(kc == 0),
                    stop=(kc == KC - 1),
                )
            nc.vector.tensor_copy(vtp[mt // 2][:, (mt % 2):(mt % 2) + 1, :], vp[:])

        # ---- main attention loop ---------------------------------------
        # Software-pipelined one pair ahead: the score matmuls + exp for
        # pair i+1 are emitted before the feat/sums matmuls of pair i, so
        # the in-order PE queue never head-of-line blocks on the fc PSUM
        # drain at nt boundaries, and ACT always has the next stp ready.
        def emit_scores(nt, g):
            n0 = nt * NT
            stp = ps_st.tile([P, 2, NT], F32, tag="stp", name=f"stp{nt}_{g}")
            for j in range(2):
                mt = 2 * g + j
                nc.tensor.matmul(
                    stp[:, j:j + 1, :],
                    lhsT=k4[32 * j:32 * j + 32, mt * P:(mt + 1) * P],
                    rhs=q4[32 * j:32 * j + 32, n0:n0 + NT],
                    start=True,
                    stop=True,
                    tile_position=(32 * j, 0),
                )
            se = pse.tile([P, 2, NT], FP8, tag="se", name=f"se{nt}_{g}")
            nc.scalar.activation(se[:], stp[:], AF.Exp, bias=ebias[:])
            return se

        seq = [(nt, g) for nt in range(N_NT) for g in range(NPAIR)]
        se_q = {seq[0]: emit_scores(*seq[0])}
        fc = sm = None
        for idx, (nt, g) in enumerate(seq):
            if g == 0:
                fc = [ps_feat.tile([P, NT], F32, tag="feat", name=f"fc{nt}_{i}")
                      for i in range(2)]
                sm = ps_sums.tile([P, NT], F32, tag="sums")
            if idx + 1 < len(seq):
                se_q[seq[idx + 1]] = emit_scores(*seq[idx + 1])
            se = se_q.pop((nt, g))
            first = g == 0
            last = g == NPAIR - 1
            for h in range(2):
                nc.tensor.matmul(
                    fc[h][:],
                    lhsT=vtp[g][:, :, h * P:(h + 1) * P],
                    rhs=se[:],
                    start=first, stop=last,
                    perf_mode=DR,
                )
            nc.tensor.matmul(
                sm[:], lhsT=ones2[:], rhs=se[:],
                start=first, stop=last,
                perf_mode=DR,
            )
            if last:
                n0 = nt * NT
                rcb = pout.tile([P, NT], F32, tag="rcb")
                nc.vector.reciprocal_approx_fast(out=rcb[:], in_=sm[:])
                for c in range(2):
                    tmp = pout.tile([P, NT], F32, tag="tmp")
                    nc.vector.tensor_tensor(tmp[:], fc[c][:], rcb[:], OP.mult)
                    ot = pout.tile([P, NT], F32, tag="ot")
                    nc.vector.tensor_scalar(ot[:], tmp[:], bv_t[c][:], None,
                                            OP.add)
                    nc.sync.dma_start(out[c * P:(c + 1) * P, n0:n0 + NT], ot[:])


_BUILT = None


def _build():
    global _BUILT
    if _BUILT is not None:
        return _BUILT
    nc = bacc.Bacc("TRN2", target_bir_lowering=False, debug=False)
    io = {
        "d": nc.dram_tensor("d", [C, HW], BF16, kind="ExternalInput"),
        "dq": nc.dram_tensor("dq", [C, NQ], BF16, kind="ExternalInput"),
        "r": nc.dram_tensor("r", [C, HW], BF16, kind="ExternalInput"),
        "wqt4": nc.dram_tensor("wqt4", [C, P], BF16, kind="ExternalInput"),
        "wkt4": nc.dram_tensor("wkt4", [C, P], BF16, kind="ExternalInput"),
        "wvt": nc.dram_tensor("wvt", [C, C], BF16, kind="ExternalInput"),
        "bq4": nc.dram_tensor("bq4", [P, 1], F32, kind="ExternalInput"),
        "bk4": nc.dram_tensor("bk4", [P, 1], F32, kind="ExternalInput"),
        "bv2": nc.dram_tensor("bv2", [C, 1], F32, kind="ExternalInput"),
        "out": nc.dram_tensor("out", [C, NQ], F32, kind="ExternalOutput"),
    }
    with tile.TileContext(nc) as tc:
        _emit(tc, io)
    nc.compile()
    _BUILT = nc
    return nc


def _in_maps(rgb, depth, Wq, bq, Wk, bk, Wv, bv):
    f = np.float32
    bf = ml_dtypes.bfloat16
    d_all = np.ascontiguousarray(depth.reshape(B, C, HW)).astype(bf)
    r_all = np.ascontiguousarray(rgb.reshape(B, C, HW)).astype(bf)
    wqt4 = np.ascontiguousarray(np.tile(np.asarray(Wq, f).T, (1, 4))).astype(bf)
    wkt4 = np.ascontiguousarray(np.tile(np.asarray(Wk, f).T, (1, 4))).astype(bf)
    wvt = np.ascontiguousarray(np.asarray(Wv, f).T).astype(bf)
    bq4 = np.ascontiguousarray(np.tile(np.asarray(bq, f), 4).reshape(P, 1))
    bk4 = np.ascontiguousarray(np.tile(np.asarray(bk, f), 4).reshape(P, 1))
    bv2 = np.ascontiguousarray(np.asarray(bv, f).reshape(C, 1))
    maps = []
    for core in range(8):
        b, half = core // 2, core % 2
        maps.append({
            "d": d_all[b],
            "dq": np.ascontiguousarray(d_all[b][:, half * NQ:(half + 1) * NQ]),
            "r": r_all[b],
            "wqt4": wqt4, "wkt4": wkt4, "wvt": wvt,
            "bq4": bq4, "bk4": bk4, "bv2": bv2,
        })
    return maps


def kernel(rgb, depth, Wq, bq, Wk, bk, Wv, bv, **run_kwargs):
    nc = _build()
    maps = _in_maps(rgb, depth, Wq, bq, Wk, bk, Wv, bv)
    res = run_bass_kernel_spmd(nc, maps, core_ids=list(range(8)), **run_kwargs)
    results = res.results if hasattr(res, "results") else res
    out = np.empty((B, C, HW), dtype=np.float32)
    for core in range(8):
        b, half = core // 2, core % 2
        out[b][:, half * NQ:(half + 1) * NQ] = results[core]["out"]
    kernel.last_results = res
    return out.reshape(B, C, H, W)


# revision 18
# speedup vs baseline: 1.7414x; 1.0795x over previous
"""CrossAttention (DFFNet) Trainium2 Bass kernel.

Shapes (hardcoded): rgb/depth [4, 256, 64, 64] f32; Wq/Wk [32, 256]; Wv [256, 256].

    q = Wq @ d + bq          [B, 32, 4096]
    k = Wk @ d + bk          [B, 32, 4096]
    v = Wv @ r + bv          [B, 256, 4096]
    scores = q^T k           [B, 4096, 4096], softmax over keys (last dim)
    feat = v @ mask^T        [B, 256, 4096]

Sharding: 8 cores = 4 batches x 2 query-halves (2048 queries each). Each core
gets full depth/rgb for its batch (keys/values span all 4096 tokens) plus its
query-half slice of depth.

Device layout: scores are computed TRANSPOSED, st[m, n] (keys m on partitions,
queries n free) so the feat matmul needs no transposes:
  - v^T[m, c] is produced directly by  r-slice^T @ Wv^T.
  - feat[c, n] = sum_m v^T[m, c] * exp(st[m, n]) / S[n].
  - softmax denominator S[n] = sum_m exp(st[m,n]) via ones-lhsT matmul.
The PE array is the bottleneck (99% busy), so the exp tiles and v^T tiles are
stored as fp8-e4m3 and the feat + sums matmuls run in DoubleRow perf mode
(two 128-deep key tiles contracted per instruction at 0.5 cycles/row), halving
their PE streaming time. exp is computed with a built-in bias of -ln(8)
(out = exp(st)/8) to center values in fp8e4's range [2^-9, 240]; the scale
cancels exactly in the softmax normalization since the denominator uses the
same scaled tiles. Scores stay bf16 (K=32, output-bound, fp8 wouldn't help).

Inputs arrive from the host pre-cast to bf16 (halves DMA bytes and removes
all on-chip f32->bf16 staging casts). Normalization: 1/S via fast reciprocal
(f32), cast to fp16, broadcast to 128 partitions via a K=1 fp16 matmul.
"""

import math

import numpy as np
import ml_dtypes

import concourse.bacc as bacc
import concourse.bass as bass
import concourse.mybir as mybir
import concourse.tile as tile
from concourse.bass_utils import run_bass_kernel_spmd

B, C, H, W = 4, 256, 64, 64
HW = H * W            # 4096
CQK = 32
P = 128
NQ = HW // 2          # 2048 queries per core
NT = 512              # query tile
N_NT = NQ // NT       # 4
MT = HW // P          # 32 key tiles
NPAIR = MT // 2       # 16 DoubleRow key-tile pairs
KC = C // P           # 2 contraction tiles for the projections

F32 = mybir.dt.float32
F16 = mybir.dt.float16
BF16 = mybir.dt.bfloat16
FP8 = mybir.dt.float8e4
AF = mybir.ActivationFunctionType
OP = mybir.AluOpType
DR = mybir.MatmulPerfMode.DoubleRow

EXP_BIAS = -math.log(8.0)   # exp(st)/8: keeps weights in fp8e4 normal range


def _emit(tc, io):
    nc = tc.nc
    d = io["d"].ap()          # [256, 4096] bf16 depth (keys source)
    dq = io["dq"].ap()        # [256, 2048] bf16 depth query-half
    r = io["r"].ap()          # [256, 4096] bf16 rgb (values source)
    wqt4 = io["wqt4"].ap()    # [256, 128] bf16 = tile(Wq.T, (1,4))
    wkt4 = io["wkt4"].ap()    # [256, 128] bf16
    wvt = io["wvt"].ap()      # [256, 256] bf16 = Wv.T
    bq4 = io["bq4"].ap()      # [128, 1] f32 = tile(bq, 4)
    bk4 = io["bk4"].ap()      # [128, 1] f32
    bv2 = io["bv2"].ap()      # [256, 1] f32
    out = io["out"].ap()      # [256, 2048] f32

    from contextlib import ExitStack

    with ExitStack() as ctx:
        pw = ctx.enter_context(tc.tile_pool(name="weights", bufs=1))
        pin = ctx.enter_context(tc.tile_pool(name="inputs", bufs=1))
        pqk = ctx.enter_context(tc.tile_pool(name="qk", bufs=1))
        pvt = ctx.enter_context(tc.tile_pool(name="vt", bufs=1))
        pse = ctx.enter_context(tc.tile_pool(name="stexp", bufs=3))
        psmall = ctx.enter_context(tc.tile_pool(name="small", bufs=2))
        pout = ctx.enter_context(tc.tile_pool(name="outsb", bufs=4))
        ps_st = ctx.enter_context(
            tc.tile_pool(name="ps_st", bufs=2, space=bass.MemorySpace.PSUM))
        ps_feat = ctx.enter_context(
            tc.tile_pool(name="ps_feat", bufs=2, space=bass.MemorySpace.PSUM))
        ps_sums = ctx.enter_context(
            tc.tile_pool(name="ps_sums", bufs=2, space=bass.MemorySpace.PSUM))

        # ---- constants / weights (inputs already bf16: direct DMA) -----
        wq_t, wk_t, wv_t, bv_t = [], [], [], []
        for kc in range(KC):
            t = pw.tile([P, P], BF16, tag=f"wq{kc}")
            nc.sync.dma_start(t[:], wqt4[kc * P:(kc + 1) * P, :])
            wq_t.append(t)
            t = pw.tile([P, P], BF16, tag=f"wk{kc}")
            nc.sync.dma_start(t[:], wkt4[kc * P:(kc + 1) * P, :])
            wk_t.append(t)
            t = pw.tile([P, C], BF16, tag=f"wv{kc}")
            nc.sync.dma_start(t[:], wvt[kc * P:(kc + 1) * P, :])
            wv_t.append(t)
            t = pw.tile([P, 1], F32, tag=f"bv{kc}")
            nc.sync.dma_start(t[:], bv2[kc * P:(kc + 1) * P, :])
            bv_t.append(t)
        bq_sb = pw.tile([P, 1], F32, tag="bq")
        nc.sync.dma_start(bq_sb[:], bq4[:])
        bk_sb = pw.tile([P, 1], F32, tag="bk")
        nc.sync.dma_start(bk_sb[:], bk4[:])
        # Full-width ones lhsT: dual-fp8 LDWEIGHTS requires col_grp=0xf (all
        # 128 PE columns), so a [128,2,1] ones vector is illegal. The [128,2,128]
        # form also lands S[n] pre-broadcast on all 128 PSUM partitions, which
        # removes the separate K=1 broadcast matmul from the critical path.
        ones2 = pw.tile([P, 2, P], FP8, tag="ones2")
        nc.vector.memset(ones2[:], 1.0)
        ebias = pw.tile([P, 1], F32, tag="ebias")
        nc.vector.memset(ebias[:], EXP_BIAS)

        # ---- inputs (per-chunk tiles so projections start early) -------
        # d (k-proj input) on the fast gpsimd ring, kc-interleaved so each
        # k-proj quarter's operand pair lands together; dq + r[kc0] on sync;
        # r[kc1] on gpsimd after d. The scalar/Activation queue stays clear
        # of descriptors so exp activations are never queued behind DMA.
        def _tile_of(dram_ap, kc, ch, pref, eng):
            t = pin.tile([P, 1024], BF16, tag=f"{pref}{kc}_{ch}",
                         name=f"{pref}{kc}_{ch}")
            eng.dma_start(
                t[:], dram_ap[kc * P:(kc + 1) * P, ch * 1024:(ch + 1) * 1024])
            return t

        d_sb = [[None] * 4 for _ in range(KC)]
        for ch in range(4):
            for kc in range(KC):
                d_sb[kc][ch] = _tile_of(d, kc, ch, "d", nc.gpsimd)
        dq_sb = [[_tile_of(dq, kc, ch, "dq", nc.sync) for ch in range(2)]
                 for kc in range(KC)]
        r_sb = [[None] * 4 for _ in range(KC)]
        for ch in range(4):
            r_sb[0][ch] = _tile_of(r, 0, ch, "r", nc.sync)
        for ch in range(4):
            r_sb[1][ch] = _tile_of(r, 1, ch, "r", nc.gpsimd)

        # ---- k / q projections (4x-replicated layouts) -----------------
        # k4[32j + o, m] = k[o, m];  q4[32j + o, n] = q[o, n]
        k4 = pqk.tile([P, HW], BF16, tag="k4")
        for qtr in range(4):
            kp = ps_st.tile([P, 2, NT], F32, tag="stp", name=f"kp{qtr}")
            for sub in range(2):
                for kc in range(KC):
                    nc.tensor.matmul(
                        kp[:, sub:sub + 1, :],
                        lhsT=wk_t[kc][:],
                        rhs=d_sb[kc][qtr][:, sub * NT:(sub + 1) * NT],
                        start=(kc == 0),
                        stop=(kc == KC - 1),
                    )
                nc.vector.tensor_scalar(
                    k4[:, qtr * 1024 + sub * NT:qtr * 1024 + (sub + 1) * NT],
                    kp[:, sub:sub + 1, :], bk_sb[:], None, OP.add,
                )
        q4 = pqk.tile([P, NQ], BF16, tag="q4")
        for half in range(2):
            qp = ps_st.tile([P, 2, NT], F32, tag="stp", name=f"qp{half}")
            for sub in range(2):
                for kc in range(KC):
                    nc.tensor.matmul(
                        qp[:, sub:sub + 1, :],
                        lhsT=wq_t[kc][:],
                        rhs=dq_sb[kc][half][:, sub * NT:(sub + 1) * NT],
                        start=(kc == 0),
                        stop=(kc == KC - 1),
                    )
                nc.vector.tensor_scalar(
                    q4[:, half * 1024 + sub * NT:half * 1024 + (sub + 1) * NT],
                    qp[:, sub:sub + 1, :], bq_sb[:], None, OP.add,
                )

        # ---- v^T projection: vtp[g][p, i, c] = v[c, (2g+i)*128 + p] ----
        # Stored fp8 in DoubleRow pair layout (no bias; bias added at the end).
        vtp = []
        for g in range(NPAIR):
            t = pvt.tile([P, 2, C], FP8, tag=f"vt{g}")
            vtp.append(t)
        # vp tiles rotate through the fc banks AND the sums banks (all idle
        # until the main loop), giving a 4-deep pipeline so the matmul ->
        # fp8-cast ping-pong never serializes on a single PSUM bank.
        for mt in range(MT):
            pool, tag = ((ps_feat, "feat"), (ps_sums, "sums"))[mt % 2]
            vp = pool.tile([P, C], F32, tag=tag, name=f"vp{mt}")
            for kc in range(KC):
                nc.tensor.matmul(
                    vp[:],
                    lhsT=r_sb[kc][mt // 8][:, (mt % 8) * P:(mt % 8 + 1) * P],
                    rhs=wv_t[kc][:],
                    start=(kc == 0),
                    stop=(kc == KC - 1),
                )
            nc.vector.tensor_copy(vtp[mt // 2][:, (mt % 2):(mt % 2) + 1, :], vp[:])

        # ---- main attention loop ---------------------------------------
        # Software-pipelined one pair ahead: the score matmuls + exp for
        # pair i+1 are emitted before the feat/sums matmuls of pair i, so
        # the in-order PE queue never head-of-line blocks on the fc PSUM
        # drain at nt boundaries, and ACT always has the next stp ready.
        def emit_scores(nt, g):
            n0 = nt * NT
            stp = ps_st.tile([P, 2, NT], F32, tag="stp", name=f"stp{nt}_{g}")
            for j in range(2):
                mt = 2 * g + j
                nc.tensor.matmul(
                    stp[:, j:j + 1, :],
                    lhsT=k4[32 * j:32 * j + 32, mt * P:(mt + 1) * P],
                    rhs=q4[32 * j:32 * j + 32, n0:n0 + NT],
                    start=True,
                    stop=True,
                    tile_position=(32 * j, 0),
                )
            se = pse.tile([P, 2, NT], FP8, tag="se", name=f"se{nt}_{g}")
            nc.scalar.activation(se[:], stp[:], AF.Exp, bias=ebias[:])
            return se

        seq = [(nt, g) for nt in range(N_NT) for g in range(NPAIR)]
        se_q = {seq[0]: emit_scores(*seq[0])}
        fc = sm = None
        for idx, (nt, g) in enumerate(seq):
            if g == 0:
                fc = [ps_feat.tile([P, NT], F32, tag="feat", name=f"fc{nt}_{i}")
                      for i in range(2)]
                sm = ps_sums.tile([P, NT], F32, tag="sums")
            if idx + 1 < len(seq):
                se_q[seq[idx + 1]] = emit_scores(*seq[idx + 1])
            se = se_q.pop((nt, g))
            first = g == 0
            last = g == NPAIR - 1
            for h in range(2):
                nc.tensor.matmul(
                    fc[h][:],
                    lhsT=vtp[g][:, :, h * P:(h + 1) * P],
                    rhs=se[:],
                    start=first, stop=last,
                    perf_mode=DR,
                )
            nc.tensor.matmul(
                sm[:], lhsT=ones2[:], rhs=se[:],
                start=first, stop=last,
                perf_mode=DR,
            )
            if last:
                n0 = nt * NT
                rcb = pout.tile([P, NT], F32, tag="rcb")
                nc.vector.reciprocal_approx_fast(out=rcb[:], in_=sm[:])
                for c in range(2):
                    tmp = pout.tile([P, NT], F32, tag="tmp")
                    nc.vector.tensor_tensor(tmp[:], fc[c][:], rcb[:], OP.mult)
                    ot = pout.tile([P, NT], F32, tag="ot")
                    nc.vector.tensor_scalar(ot[:], tmp[:], bv_t[c][:], None,
                                            OP.add)
                    nc.sync.dma_start(out[c * P:(c + 1) * P, n0:n0 + NT], ot[:])


_BUILT = None


def _build():
    global _BUILT
    if _BUILT is not None:
        return _BUILT
    nc = bacc.Bacc("TRN2", target_bir_lowering=False, debug=False)
    io = {
        "d": nc.dram_tensor("d", [C, HW], BF16, kind="ExternalInput"),
        "dq": nc.dram_tensor("dq", [C, NQ], BF16, kind="ExternalInput"),
        "r": nc.dram_tensor("r", [C, HW], BF16, kind="ExternalInput"),
        "wqt4": nc.dram_tensor("wqt4", [C, P], BF16, kind="ExternalInput"),
        "wkt4": nc.dram_tensor("wkt4", [C, P], BF16, kind="ExternalInput"),
        "wvt": nc.dram_tensor("wvt", [C, C], BF16, kind="ExternalInput"),
        "bq4": nc.dram_tensor("bq4", [P, 1], F32, kind="ExternalInput"),
        "bk4": nc.dram_tensor("bk4", [P, 1], F32, kind="ExternalInput"),
        "bv2": nc.dram_tensor("bv2", [C, 1], F32, kind="ExternalInput"),
        "out": nc.dram_tensor("out", [C, NQ], F32, kind="ExternalOutput"),
    }
    with tile.TileContext(nc) as tc:
        _emit(tc, io)
    nc.compile()
    _BUILT = nc
    return nc


def _in_maps(rgb, depth, Wq, bq, Wk, bk, Wv, bv):
    f = np.float32
    bf = ml_dtypes.bfloat16
    d_all = np.ascontiguousarray(depth.reshape(B, C, HW)).astype(bf)
    r_all = np.ascontiguousarray(rgb.reshape(B, C, HW)).astype(bf)
    wqt4 = np.ascontiguousarray(np.tile(np.asarray(Wq, f).T, (1, 4))).astype(bf)
    wkt4 = np.ascontiguousarray(np.tile(np.asarray(Wk, f).T, (1, 4))).astype(bf)
    wvt = np.ascontiguousarray(np.asarray(Wv, f).T).astype(bf)
    bq4 = np.ascontiguousarray(np.tile(np.asarray(bq, f), 4).reshape(P, 1))
    bk4 = np.ascontiguousarray(np.tile(np.asarray(bk, f), 4).reshape(P, 1))
    bv2 = np.ascontiguousarray(np.asarray(bv, f).reshape(C, 1))
    maps = []
    for core in range(8):
        b, half = core // 2, core % 2
        maps.append({
            "d": d_all[b],
            "dq": np.ascontiguousarray(d_all[b][:, half * NQ:(half + 1) * NQ]),
            "r": r_all[b],
            "wqt4": wqt4, "wkt4": wkt4, "wvt": wvt,
            "bq4": bq4, "bk4": bk4, "bv2": bv2,
        })
    return maps


def kernel(rgb, depth, Wq, bq, Wk, bk, Wv, bv, **run_kwargs):
    nc = _build()
    maps = _in_maps(rgb, depth, Wq, bq, Wk, bk, Wv, bv)
    res = run_bass_kernel_spmd(nc, maps, core_ids=list(range(8)), **run_kwargs)
    results = res.results if hasattr(res, "results") else res
    out = np.empty((B, C, HW), dtype=np.float32)
    for core in range(8):
        b, half = core // 2, core % 2
        out[b][:, half * NQ:(half + 1) * NQ] = results[core]["out"]
    kernel.last_results = res
    return out.reshape(B, C, H, W)


# revision 24
# speedup vs baseline: 1.8342x; 1.0533x over previous
"""CrossAttention (DFFNet) Trainium2 Bass kernel.

Shapes (hardcoded): rgb/depth [4, 256, 64, 64] f32; Wq/Wk [32, 256]; Wv [256, 256].

    q = Wq @ d + bq          [B, 32, 4096]
    k = Wk @ d + bk          [B, 32, 4096]
    v = Wv @ r + bv          [B, 256, 4096]
    scores = q^T k           [B, 4096, 4096], softmax over keys (last dim)
    feat = v @ mask^T        [B, 256, 4096]

Sharding: 8 cores = 4 batches x 2 query-halves (2048 queries each). Each core
gets full depth/rgb for its batch (keys/values span all 4096 tokens) plus its
query-half slice of depth.

Device layout: scores are computed TRANSPOSED, st[m, n] (keys m on partitions,
queries n free) so the feat matmul needs no transposes:
  - v^T[m, c] is produced directly by  r-slice^T @ Wv^T.
  - feat[c, n] = sum_m v^T[m, c] * exp(st[m, n]) / S[n].
  - softmax denominator S[n] = sum_m exp(st[m,n]) via ones-lhsT matmul.
The PE array is the bottleneck (99% busy), so the exp tiles and v^T tiles are
stored as fp8-e4m3 and the feat + sums matmuls run in DoubleRow perf mode
(two 128-deep key tiles contracted per instruction at 0.5 cycles/row), halving
their PE streaming time. exp is computed with a built-in bias of -ln(8)
(out = exp(st)/8) to center values in fp8e4's range [2^-9, 240]; the scale
cancels exactly in the softmax normalization since the denominator uses the
same scaled tiles. Scores stay bf16 (K=32, output-bound, fp8 wouldn't help).

Inputs arrive from the host pre-cast to bf16 (halves DMA bytes and removes
all on-chip f32->bf16 staging casts). Normalization: 1/S via fast reciprocal
(f32), cast to fp16, broadcast to 128 partitions via a K=1 fp16 matmul.
"""

import math

import numpy as np
import ml_dtypes

import concourse.bacc as bacc
import concourse.bass as bass
import concourse.mybir as mybir
import concourse.tile as tile
from concourse.bass_utils import run_bass_kernel_spmd

B, C, H, W = 4, 256, 64, 64
HW = H * W            # 4096
CQK = 32
P = 128
NQ = HW // 2          # 2048 queries per core
NT = 512              # query tile
N_NT = NQ // NT       # 4
MT = HW // P          # 32 key tiles
NPAIR = MT // 2       # 16 DoubleRow key-tile pairs
KC = C // P           # 2 contraction tiles for the projections

F32 = mybir.dt.float32
F16 = mybir.dt.float16
BF16 = mybir.dt.bfloat16
FP8 = mybir.dt.float8e4
AF = mybir.ActivationFunctionType
OP = mybir.AluOpType
DR = mybir.MatmulPerfMode.DoubleRow

EXP_BIAS = -math.log(8.0)   # exp(st)/8: keeps weights in fp8e4 normal range


def _emit(tc, io):
    nc = tc.nc
    d = io["d"].ap()          # [256, 4096] bf16 depth, query-half rotated first
    r = io["r"].ap()          # [256, 4096] bf16 rgb, same rotation as d
    wqt4 = io["wqt4"].ap()    # [256, 128] bf16 = tile(Wq.T, (1,4))
    wkt4 = io["wkt4"].ap()    # [256, 128] bf16
    wvt = io["wvt"].ap()      # [256, 256] bf16 = Wv.T
    bq4 = io["bq4"].ap()      # [128, 1] f32 = tile(bq, 4)
    bk4 = io["bk4"].ap()      # [128, 1] f32
    bv2 = io["bv2"].ap()      # [256, 1] f32
    out = io["out"].ap()      # [256, 2048] f32

    from contextlib import ExitStack

    with ExitStack() as ctx:
        pw = ctx.enter_context(tc.tile_pool(name="weights", bufs=1))
        pin = ctx.enter_context(tc.tile_pool(name="inputs", bufs=1))
        pqk = ctx.enter_context(tc.tile_pool(name="qk", bufs=1))
        pvt = ctx.enter_context(tc.tile_pool(name="vt", bufs=1))
        pse = ctx.enter_context(tc.tile_pool(name="stexp", bufs=4))
        psmall = ctx.enter_context(tc.tile_pool(name="small", bufs=2))
        pout = ctx.enter_context(tc.tile_pool(name="outsb", bufs=4))
        ps_st = ctx.enter_context(
            tc.tile_pool(name="ps_st", bufs=2, space=bass.MemorySpace.PSUM))
        ps_feat = ctx.enter_context(
            tc.tile_pool(name="ps_feat", bufs=2, space=bass.MemorySpace.PSUM))
        ps_sums = ctx.enter_context(
            tc.tile_pool(name="ps_sums", bufs=2, space=bass.MemorySpace.PSUM))

        # ---- constants / weights (inputs already bf16: direct DMA) -----
        wq_t, wk_t, wv_t, bv_t = [], [], [], []
        for kc in range(KC):
            t = pw.tile([P, P], BF16, tag=f"wq{kc}")
            nc.sync.dma_start(t[:], wqt4[kc * P:(kc + 1) * P, :])
            wq_t.append(t)
            t = pw.tile([P, P], BF16, tag=f"wk{kc}")
            nc.sync.dma_start(t[:], wkt4[kc * P:(kc + 1) * P, :])
            wk_t.append(t)
            t = pw.tile([P, C], BF16, tag=f"wv{kc}")
            nc.sync.dma_start(t[:], wvt[kc * P:(kc + 1) * P, :])
            wv_t.append(t)
            t = pw.tile([P, 1], F32, tag=f"bv{kc}")
            nc.sync.dma_start(t[:], bv2[kc * P:(kc + 1) * P, :])
            bv_t.append(t)
        bq_sb = pw.tile([P, 1], F32, tag="bq")
        nc.sync.dma_start(bq_sb[:], bq4[:])
        bk_sb = pw.tile([P, 1], F32, tag="bk")
        nc.sync.dma_start(bk_sb[:], bk4[:])
        # Full-width ones lhsT: dual-fp8 LDWEIGHTS requires col_grp=0xf (all
        # 128 PE columns), so a [128,2,1] ones vector is illegal. The [128,2,128]
        # form also lands S[n] pre-broadcast on all 128 PSUM partitions, which
        # removes the separate K=1 broadcast matmul from the critical path.
        ones2 = pw.tile([P, 2, P], FP8, tag="ones2")
        nc.vector.memset(ones2[:], 1.0)
        ebias = pw.tile([P, 1], F32, tag="ebias")
        nc.vector.memset(ebias[:], EXP_BIAS)

        # ---- inputs (per-chunk tiles so projections start early) -------
        # d and r are column-rotated per core on the host so this core's
        # query tokens are d's FIRST 2048 columns (q-proj reads d chunks 0-1
        # directly; no separate dq input). Key/value order is a free
        # permutation: softmax and feat both just sum over keys.
        # DMA rings: d + r[kc1] on gpsimd (fast ring), r[kc0] on sync after
        # the weights. The scalar/Activation queue stays clear of
        # descriptors so exp activations are never queued behind DMA.
        def _tile_of(dram_ap, kc, ch, pref, eng):
            t = pin.tile([P, 1024], BF16, tag=f"{pref}{kc}_{ch}",
                         name=f"{pref}{kc}_{ch}")
            eng.dma_start(
                t[:], dram_ap[kc * P:(kc + 1) * P, ch * 1024:(ch + 1) * 1024])
            return t

        d_sb = [[None] * 4 for _ in range(KC)]
        for ch in range(4):
            for kc in range(KC):
                d_sb[kc][ch] = _tile_of(d, kc, ch, "d", nc.gpsimd)
        r_sb = [[None] * 4 for _ in range(KC)]
        for ch in range(4):
            r_sb[0][ch] = _tile_of(r, 0, ch, "r", nc.sync)
        for ch in range(4):
            r_sb[1][ch] = _tile_of(r, 1, ch, "r", nc.gpsimd)

        # ---- q projection first (needs only d chunks 0-1) ---------------
        # q4[32j + o, n] = q[o, n] (4x-replicated for score row packing)
        q4 = pqk.tile([P, NQ], BF16, tag="q4")
        for half in range(2):
            qp = ps_st.tile([P, 2, NT], F32, tag="stp", name=f"qp{half}")
            for sub in range(2):
                for kc in range(KC):
                    nc.tensor.matmul(
                        qp[:, sub:sub + 1, :],
                        lhsT=wq_t[kc][:],
                        rhs=d_sb[kc][half][:, sub * NT:(sub + 1) * NT],
                        start=(kc == 0),
                        stop=(kc == KC - 1),
                    )
                nc.vector.tensor_scalar(
                    q4[:, half * 1024 + sub * NT:half * 1024 + (sub + 1) * NT],
                    qp[:, sub:sub + 1, :], bq_sb[:], None, OP.add,
                )

        # ---- v^T projection: vtp[g][p, i, c] = v[c, (2g+i)*128 + p] ----
        # Stored fp8 in DoubleRow pair layout (no bias; bias added at the
        # end). Emitted before k-proj so the fc/sums PSUM banks it borrows
        # are drained well before the main loop starts. The fp8 casts split
        # between DVE and the (still idle) ACT engine.
        vtp = []
        for g in range(NPAIR):
            t = pvt.tile([P, 2, C], FP8, tag=f"vt{g}")
            vtp.append(t)
        for mt in range(MT):
            pool, tag = ((ps_feat, "feat"), (ps_sums, "sums"))[mt % 2]
            vp = pool.tile([P, C], F32, tag=tag, name=f"vp{mt}")
            for kc in range(KC):
                nc.tensor.matmul(
                    vp[:],
                    lhsT=r_sb[kc][mt // 8][:, (mt % 8) * P:(mt % 8 + 1) * P],
                    rhs=wv_t[kc][:],
                    start=(kc == 0),
                    stop=(kc == KC - 1),
                )
            dst = vtp[mt // 2][:, (mt % 2):(mt % 2) + 1, :]
            if mt % 4 < 2:
                nc.vector.tensor_copy(dst, vp[:])
            else:
                nc.scalar.copy(dst, vp[:])

        # ---- k projection: k4[32j + o, m] = k[o, m] ---------------------
        k4 = pqk.tile([P, HW], BF16, tag="k4")
        for qtr in range(4):
            kp = ps_st.tile([P, 2, NT], F32, tag="stp", name=f"kp{qtr}")
            for sub in range(2):
                for kc in range(KC):
                    nc.tensor.matmul(
                        kp[:, sub:sub + 1, :],
                        lhsT=wk_t[kc][:],
                        rhs=d_sb[kc][qtr][:, sub * NT:(sub + 1) * NT],
                        start=(kc == 0),
                        stop=(kc == KC - 1),
                    )
                nc.vector.tensor_scalar(
                    k4[:, qtr * 1024 + sub * NT:qtr * 1024 + (sub + 1) * NT],
                    kp[:, sub:sub + 1, :], bk_sb[:], None, OP.add,
                )

        # ---- main attention loop ---------------------------------------
        # Software-pipelined one pair ahead: the score matmuls + exp for
        # pair i+1 are emitted before the feat/sums matmuls of pair i, so
        # the in-order PE queue never head-of-line blocks on the fc PSUM
        # drain at nt boundaries, and ACT always has the next stp ready.
        def emit_scores(nt, g):
            n0 = nt * NT
            stp = ps_st.tile([P, 2, NT], F32, tag="stp", name=f"stp{nt}_{g}")
            for j in range(2):
                mt = 2 * g + j
                nc.tensor.matmul(
                    stp[:, j:j + 1, :],
                    lhsT=k4[32 * j:32 * j + 32, mt * P:(mt + 1) * P],
                    rhs=q4[32 * j:32 * j + 32, n0:n0 + NT],
                    start=True,
                    stop=True,
                    tile_position=(32 * j, 0),
                )
            se = pse.tile([P, 2, NT], FP8, tag="se", name=f"se{nt}_{g}")
            nc.scalar.activation(se[:], stp[:], AF.Exp, bias=ebias[:])
            return se

        seq = [(nt, g) for nt in range(N_NT) for g in range(NPAIR)]
        se_q = {seq[0]: emit_scores(*seq[0])}
        fc = sm = None
        for idx, (nt, g) in enumerate(seq):
            if g == 0:
                fc = [ps_feat.tile([P, NT], F32, tag="feat", name=f"fc{nt}_{i}")
                      for i in range(2)]
                sm = ps_sums.tile([P, NT], F32, tag="sums")
            if idx + 1 < len(seq):
                se_q[seq[idx + 1]] = emit_scores(*seq[idx + 1])
            se = se_q.pop((nt, g))
            first = g == 0
            last = g == NPAIR - 1
            for h in range(2):
                nc.tensor.matmul(
                    fc[h][:],
                    lhsT=vtp[g][:, :, h * P:(h + 1) * P],
                    rhs=se[:],
                    start=first, stop=last,
                    perf_mode=DR,
                )
            nc.tensor.matmul(
                sm[:], lhsT=ones2[:], rhs=se[:],
                start=first, stop=last,
                perf_mode=DR,
            )
            if last:
                n0 = nt * NT
                rcb = pout.tile([P, NT], F32, tag="rcb")
                nc.vector.reciprocal_approx_fast(out=rcb[:], in_=sm[:])
                for c in range(2):
                    tmp = pout.tile([P, NT], F32, tag="tmp")
                    nc.vector.tensor_tensor(tmp[:], fc[c][:], rcb[:], OP.mult)
                    ot = pout.tile([P, NT], F32, tag="ot")
                    nc.vector.tensor_scalar(ot[:], tmp[:], bv_t[c][:], None,
                                            OP.add)
                    nc.sync.dma_start(out[c * P:(c + 1) * P, n0:n0 + NT], ot[:])


_BUILT = None


def _build():
    global _BUILT
    if _BUILT is not None:
        return _BUILT
    nc = bacc.Bacc("TRN2", target_bir_lowering=False, debug=False)
    io = {
        "d": nc.dram_tensor("d", [C, HW], BF16, kind="ExternalInput"),
        "r": nc.dram_tensor("r", [C, HW], BF16, kind="ExternalInput"),
        "wqt4": nc.dram_tensor("wqt4", [C, P], BF16, kind="ExternalInput"),
        "wkt4": nc.dram_tensor("wkt4", [C, P], BF16, kind="ExternalInput"),
        "wvt": nc.dram_tensor("wvt", [C, C], BF16, kind="ExternalInput"),
        "bq4": nc.dram_tensor("bq4", [P, 1], F32, kind="ExternalInput"),
        "bk4": nc.dram_tensor("bk4", [P, 1], F32, kind="ExternalInput"),
        "bv2": nc.dram_tensor("bv2", [C, 1], F32, kind="ExternalInput"),
        "out": nc.dram_tensor("out", [C, NQ], F32, kind="ExternalOutput"),
    }
    with tile.TileContext(nc) as tc:
        _emit(tc, io)
    nc.compile()
    _BUILT = nc
    return nc


def _in_maps(rgb, depth, Wq, bq, Wk, bk, Wv, bv):
    f = np.float32
    bf = ml_dtypes.bfloat16
    d_all = np.ascontiguousarray(depth.reshape(B, C, HW)).astype(bf)
    r_all = np.ascontiguousarray(rgb.reshape(B, C, HW)).astype(bf)
    wqt4 = np.ascontiguousarray(np.tile(np.asarray(Wq, f).T, (1, 4))).astype(bf)
    wkt4 = np.ascontiguousarray(np.tile(np.asarray(Wk, f).T, (1, 4))).astype(bf)
    wvt = np.ascontiguousarray(np.asarray(Wv, f).T).astype(bf)
    bq4 = np.ascontiguousarray(np.tile(np.asarray(bq, f), 4).reshape(P, 1))
    bk4 = np.ascontiguousarray(np.tile(np.asarray(bk, f), 4).reshape(P, 1))
    bv2 = np.ascontiguousarray(np.asarray(bv, f).reshape(C, 1))
    maps = []
    for core in range(8):
        b, half = core // 2, core % 2
        if half == 0:
            d_c, r_c = d_all[b], r_all[b]
        else:
            # Rotate so this core's query tokens are the first NQ columns;
            # key/value column order is a free permutation of the reduction.
            d_c = np.ascontiguousarray(np.roll(d_all[b], -NQ, axis=1))
            r_c = np.ascontiguousarray(np.roll(r_all[b], -NQ, axis=1))
        maps.append({
            "d": d_c,
            "r": r_c,
            "wqt4": wqt4, "wkt4": wkt4, "wvt": wvt,
            "bq4": bq4, "bk4": bk4, "bv2": bv2,
        })
    return maps


def kernel(rgb, depth, Wq, bq, Wk, bk, Wv, bv, **run_kwargs):
    nc = _build()
    maps = _in_maps(rgb, depth, Wq, bq, Wk, bk, Wv, bv)
    res = run_bass_kernel_spmd(nc, maps, core_ids=list(range(8)), **run_kwargs)
    results = res.results if hasattr(res, "results") else res
    out = np.empty((B, C, HW), dtype=np.float32)
    for core in range(8):
        b, half = core // 2, core % 2
        out[b][:, half * NQ:(half + 1) * NQ] = results[core]["out"]
    kernel.last_results = res
    return out.reshape(B, C, H, W)


# revision 30
# speedup vs baseline: 1.8797x; 1.0248x over previous
"""CrossAttention (DFFNet) Trainium2 Bass kernel.

Shapes (hardcoded): rgb/depth [4, 256, 64, 64] f32; Wq/Wk [32, 256]; Wv [256, 256].

    q = Wq @ d + bq          [B, 32, 4096]
    k = Wk @ d + bk          [B, 32, 4096]
    v = Wv @ r + bv          [B, 256, 4096]
    scores = q^T k           [B, 4096, 4096], softmax over keys (last dim)
    feat = v @ mask^T        [B, 256, 4096]

Sharding: 8 cores = 4 batches x 2 query-halves (2048 queries each). Each core
gets full depth/rgb for its batch (keys/values span all 4096 tokens) plus its
query-half slice of depth.

Device layout: scores are computed TRANSPOSED, st[m, n] (keys m on partitions,
queries n free) so the feat matmul needs no transposes:
  - v^T[m, c] is produced directly by  r-slice^T @ Wv^T.
  - feat[c, n] = sum_m v^T[m, c] * exp(st[m, n]) / S[n].
  - softmax denominator S[n] = sum_m exp(st[m,n]) via ones-lhsT matmul.
The PE array is the bottleneck (99% busy), so the exp tiles and v^T tiles are
stored as fp8-e4m3 and the feat + sums matmuls run in DoubleRow perf mode
(two 128-deep key tiles contracted per instruction at 0.5 cycles/row), halving
their PE streaming time. exp is computed with a built-in bias of -ln(8)
(out = exp(st)/8) to center values in fp8e4's range [2^-9, 240]; the scale
cancels exactly in the softmax normalization since the denominator uses the
same scaled tiles. Scores stay bf16 (K=32, output-bound, fp8 wouldn't help).

Inputs arrive from the host pre-cast to bf16 (halves DMA bytes and removes
all on-chip f32->bf16 staging casts). Normalization: 1/S via fast reciprocal
(f32), cast to fp16, broadcast to 128 partitions via a K=1 fp16 matmul.
"""

import math

import numpy as np
import ml_dtypes

import concourse.bacc as bacc
import concourse.bass as bass
import concourse.mybir as mybir
import concourse.tile as tile
from concourse.bass_utils import run_bass_kernel_spmd

B, C, H, W = 4, 256, 64, 64
HW = H * W            # 4096
CQK = 32
P = 128
NQ = HW // 2          # 2048 queries per core
NT = 512              # query tile
N_NT = NQ // NT       # 4
MT = HW // P          # 32 key tiles
NPAIR = MT // 2       # 16 DoubleRow key-tile pairs
KC = C // P           # 2 contraction tiles for the projections

F32 = mybir.dt.float32
F16 = mybir.dt.float16
BF16 = mybir.dt.bfloat16
FP8 = mybir.dt.float8e4
AF = mybir.ActivationFunctionType
OP = mybir.AluOpType
DR = mybir.MatmulPerfMode.DoubleRow

EXP_BIAS = -math.log(8.0)   # exp(st)/8: keeps weights in fp8e4 normal range


def _emit(tc, io):
    nc = tc.nc
    d = io["d"].ap()          # [256, 4096] bf16 depth, query-half rotated first
    r = io["r"].ap()          # [256, 4096] bf16 rgb, same rotation as d
    wqt4 = io["wqt4"].ap()    # [256, 128] bf16 = tile(Wq.T, (1,4))
    wkt4 = io["wkt4"].ap()    # [256, 128] bf16
    wvt = io["wvt"].ap()      # [256, 256] bf16 = Wv.T
    # all bias vectors packed in one tensor so ONE early DMA covers them:
    # col0 = tile(bq,4), col1 = tile(bk,4), col2/3 = bv[0:128]/bv[128:256]
    biasv = io["biasv"].ap()  # [128, 4] f32
    out = io["out"].ap()      # [256, 2048] f32

    from contextlib import ExitStack

    with ExitStack() as ctx:
        pw = ctx.enter_context(tc.tile_pool(name="weights", bufs=1))
        pin = ctx.enter_context(tc.tile_pool(name="inputs", bufs=1))
        pqk = ctx.enter_context(tc.tile_pool(name="qk", bufs=1))
        pvt = ctx.enter_context(tc.tile_pool(name="vt", bufs=1))
        pse = ctx.enter_context(tc.tile_pool(name="stexp", bufs=8))
        psmall = ctx.enter_context(tc.tile_pool(name="small", bufs=2))
        pout = ctx.enter_context(tc.tile_pool(name="outsb", bufs=4))
        ps_st = ctx.enter_context(
            tc.tile_pool(name="ps_st", bufs=2, space=bass.MemorySpace.PSUM))
        ps_feat = ctx.enter_context(
            tc.tile_pool(name="ps_feat", bufs=2, space=bass.MemorySpace.PSUM))
        ps_sums = ctx.enter_context(
            tc.tile_pool(name="ps_sums", bufs=2, space=bass.MemorySpace.PSUM))

        # ---- constants / weights (inputs already bf16: direct DMA) -----
        # The packed bias vector goes FIRST on the sync ring: it is consumed
        # by the earliest DVE ops, and issuing it late would alias its
        # completion semaphore with later big transfers (false dependency).
        bias_sb = pw.tile([P, 4], F32, tag="biasv")
        nc.sync.dma_start(bias_sb[:], biasv[:])
        bq_sb = bias_sb[:, 0:1]
        bk_sb = bias_sb[:, 1:2]
        bv_t = [bias_sb[:, 2:3], bias_sb[:, 3:4]]
        wq_t, wk_t, wv_t = [], [], []
        for kc in range(KC):
            t = pw.tile([P, P], BF16, tag=f"wq{kc}")
            nc.sync.dma_start(t[:], wqt4[kc * P:(kc + 1) * P, :])
            wq_t.append(t)
            t = pw.tile([P, P], BF16, tag=f"wk{kc}")
            nc.sync.dma_start(t[:], wkt4[kc * P:(kc + 1) * P, :])
            wk_t.append(t)
            t = pw.tile([P, C], BF16, tag=f"wv{kc}")
            nc.sync.dma_start(t[:], wvt[kc * P:(kc + 1) * P, :])
            wv_t.append(t)
        # Full-width ones lhsT: dual-fp8 LDWEIGHTS requires col_grp=0xf (all
        # 128 PE columns), so a [128,2,1] ones vector is illegal. The [128,2,128]
        # form also lands S[n] pre-broadcast on all 128 PSUM partitions, which
        # removes the separate K=1 broadcast matmul from the critical path.
        ones2 = pw.tile([P, 2, P], FP8, tag="ones2")
        nc.vector.memset(ones2[:], 1.0)
        ebias = pw.tile([P, 1], F32, tag="ebias")
        nc.vector.memset(ebias[:], EXP_BIAS)

        # ---- inputs (per-chunk tiles so projections start early) -------
        # d and r are column-rotated per core on the host so this core's
        # query tokens are d's FIRST 2048 columns (q-proj reads d chunks 0-1
        # directly; no separate dq input). Key/value order is a free
        # permutation: softmax and feat both just sum over keys.
        # DMA rings: d + r[kc1] on gpsimd (fast ring), r[kc0] on sync after
        # the weights. The scalar/Activation queue stays clear of
        # descriptors so exp activations are never queued behind DMA.
        def _tile_of(dram_ap, kc, ch, pref, eng):
            t = pin.tile([P, 1024], BF16, tag=f"{pref}{kc}_{ch}",
                         name=f"{pref}{kc}_{ch}")
            eng.dma_start(
                t[:], dram_ap[kc * P:(kc + 1) * P, ch * 1024:(ch + 1) * 1024])
            return t

        d_sb = [[None] * 4 for _ in range(KC)]
        for ch in range(4):
            for kc in range(KC):
                d_sb[kc][ch] = _tile_of(d, kc, ch, "d", nc.gpsimd)
        r_sb = [[None] * 4 for _ in range(KC)]
        for ch in range(4):
            r_sb[0][ch] = _tile_of(r, 0, ch, "r", nc.sync)
        for ch in range(4):
            r_sb[1][ch] = _tile_of(r, 1, ch, "r", nc.gpsimd)

        # ---- q projection first (needs only d chunks 0-1) ---------------
        # q4[32j + o, n] = q[o, n] (4x-replicated for score row packing)
        q4 = pqk.tile([P, NQ], BF16, tag="q4")
        for half in range(2):
            qp = ps_st.tile([P, 2, NT], F32, tag="stp", name=f"qp{half}")
            for sub in range(2):
                for kc in range(KC):
                    nc.tensor.matmul(
                        qp[:, sub:sub + 1, :],
                        lhsT=wq_t[kc][:],
                        rhs=d_sb[kc][half][:, sub * NT:(sub + 1) * NT],
                        start=(kc == 0),
                        stop=(kc == KC - 1),
                    )
                nc.vector.tensor_scalar(
                    q4[:, half * 1024 + sub * NT:half * 1024 + (sub + 1) * NT],
                    qp[:, sub:sub + 1, :], bq_sb, None, OP.add,
                )

        # ---- k projection: k4[32j + o, m] = k[o, m] ---------------------
        k4 = pqk.tile([P, HW], BF16, tag="k4")
        for qtr in range(4):
            kp = ps_st.tile([P, 2, NT], F32, tag="stp", name=f"kp{qtr}")
            for sub in range(2):
                for kc in range(KC):
                    nc.tensor.matmul(
                        kp[:, sub:sub + 1, :],
                        lhsT=wk_t[kc][:],
                        rhs=d_sb[kc][qtr][:, sub * NT:(sub + 1) * NT],
                        start=(kc == 0),
                        stop=(kc == KC - 1),
                    )
                nc.vector.tensor_scalar(
                    k4[:, qtr * 1024 + sub * NT:qtr * 1024 + (sub + 1) * NT],
                    kp[:, sub:sub + 1, :], bk_sb, None, OP.add,
                )

        def emit_scores(nt, g):
            n0 = nt * NT
            stp = ps_st.tile([P, 2, NT], F32, tag="stp", name=f"stp{nt}_{g}")
            for j in range(2):
                mt = 2 * g + j
                nc.tensor.matmul(
                    stp[:, j:j + 1, :],
                    lhsT=k4[32 * j:32 * j + 32, mt * P:(mt + 1) * P],
                    rhs=q4[32 * j:32 * j + 32, n0:n0 + NT],
                    start=True,
                    stop=True,
                    tile_position=(32 * j, 0),
                )
            se = pse.tile([P, 2, NT], FP8, tag="se", name=f"se{nt}_{g}")
            nc.scalar.activation(se[:], stp[:], AF.Exp, bias=ebias[:])
            return se

        seq = [(nt, g) for nt in range(N_NT) for g in range(NPAIR)]
        se_q = {}

        # ---- bank phase: v^T projection interleaved with the first LOOK
        # score/exp pairs. vtp[g][p, i, c] = v[c, (2g+i)*128 + p], stored fp8
        # in DoubleRow pair layout (no bias; bias added at the end). The vp
        # PSUM tiles rotate through the fc + sums banks (idle until the main
        # loop). Casts for the first half go to ACT (interleaving between the
        # banked exps); the second half to DVE (after the k/q bias adds) so
        # both finish just before the fc/sm allocations need the banks back.
        LOOK = 6
        SCORE_AT = {0: 0, 6: 1, 11: 2, 16: 3, 22: 4, 27: 5}
        vtp = []
        for g in range(NPAIR):
            t = pvt.tile([P, 2, C], FP8, tag=f"vt{g}")
            vtp.append(t)
        for mt in range(MT):
            if mt in SCORE_AT:
                i = SCORE_AT[mt]
                se_q[seq[i]] = emit_scores(*seq[i])
            pool, tag = ((ps_feat, "feat"), (ps_sums, "sums"))[mt % 2]
            vp = pool.tile([P, C], F32, tag=tag, name=f"vp{mt}")
            for kc in range(KC):
                nc.tensor.matmul(
                    vp[:],
                    lhsT=r_sb[kc][mt // 8][:, (mt % 8) * P:(mt % 8 + 1) * P],
                    rhs=wv_t[kc][:],
                    start=(kc == 0),
                    stop=(kc == KC - 1),
                )
            dst = vtp[mt // 2][:, (mt % 2):(mt % 2) + 1, :]
            if mt < 16:
                nc.scalar.copy(dst, vp[:])
            else:
                nc.vector.tensor_copy(dst, vp[:])

        # ---- main attention loop (scores/exp LOOK pairs ahead) ----------
        fc = sm = None
        for idx, (nt, g) in enumerate(seq):
            if g == 0:
                fc = [ps_feat.tile([P, NT], F32, tag="feat", name=f"fc{nt}_{i}")
                      for i in range(2)]
                sm = ps_sums.tile([P, NT], F32, tag="sums")
            if idx + LOOK < len(seq):
                se_q[seq[idx + LOOK]] = emit_scores(*seq[idx + LOOK])
            se = se_q.pop((nt, g))
            first = g == 0
            last = g == NPAIR - 1
            for h in range(2):
                nc.tensor.matmul(
                    fc[h][:],
                    lhsT=vtp[g][:, :, h * P:(h + 1) * P],
                    rhs=se[:],
                    start=first, stop=last,
                    perf_mode=DR,
                )
            nc.tensor.matmul(
                sm[:], lhsT=ones2[:], rhs=se[:],
                start=first, stop=last,
                perf_mode=DR,
            )
            if last:
                n0 = nt * NT
                rcb = pout.tile([P, NT], F32, tag="rcb")
                nc.vector.reciprocal_approx_fast(out=rcb[:], in_=sm[:])
                for c in range(2):
                    tmp = pout.tile([P, NT], F32, tag="tmp")
                    nc.vector.tensor_tensor(tmp[:], fc[c][:], rcb[:], OP.mult)
                    ot = pout.tile([P, NT], F32, tag="ot")
                    nc.vector.tensor_scalar(ot[:], tmp[:], bv_t[c], None,
                                            OP.add)
                    nc.sync.dma_start(out[c * P:(c + 1) * P, n0:n0 + NT], ot[:])


_BUILT = None


def _build():
    global _BUILT
    if _BUILT is not None:
        return _BUILT
    nc = bacc.Bacc("TRN2", target_bir_lowering=False, debug=False)
    io = {
        "d": nc.dram_tensor("d", [C, HW], BF16, kind="ExternalInput"),
        "r": nc.dram_tensor("r", [C, HW], BF16, kind="ExternalInput"),
        "wqt4": nc.dram_tensor("wqt4", [C, P], BF16, kind="ExternalInput"),
        "wkt4": nc.dram_tensor("wkt4", [C, P], BF16, kind="ExternalInput"),
        "wvt": nc.dram_tensor("wvt", [C, C], BF16, kind="ExternalInput"),
        "biasv": nc.dram_tensor("biasv", [P, 4], F32, kind="ExternalInput"),
        "out": nc.dram_tensor("out", [C, NQ], F32, kind="ExternalOutput"),
    }
    with tile.TileContext(nc) as tc:
        _emit(tc, io)
    nc.compile()
    _BUILT = nc
    return nc


def _in_maps(rgb, depth, Wq, bq, Wk, bk, Wv, bv):
    f = np.float32
    bf = ml_dtypes.bfloat16
    d_all = np.ascontiguousarray(depth.reshape(B, C, HW)).astype(bf)
    r_all = np.ascontiguousarray(rgb.reshape(B, C, HW)).astype(bf)
    wqt4 = np.ascontiguousarray(np.tile(np.asarray(Wq, f).T, (1, 4))).astype(bf)
    wkt4 = np.ascontiguousarray(np.tile(np.asarray(Wk, f).T, (1, 4))).astype(bf)
    wvt = np.ascontiguousarray(np.asarray(Wv, f).T).astype(bf)
    biasv = np.stack([np.tile(np.asarray(bq, f), 4),
                      np.tile(np.asarray(bk, f), 4),
                      np.asarray(bv, f)[:P],
                      np.asarray(bv, f)[P:]], axis=1)
    biasv = np.ascontiguousarray(biasv, dtype=f)
    maps = []
    for core in range(8):
        b, half = core // 2, core % 2
        if half == 0:
            d_c, r_c = d_all[b], r_all[b]
        else:
            # Rotate so this core's query tokens are the first NQ columns;
            # key/value column order is a free permutation of the reduction.
            d_c = np.ascontiguousarray(np.roll(d_all[b], -NQ, axis=1))
            r_c = np.ascontiguousarray(np.roll(r_all[b], -NQ, axis=1))
        maps.append({
            "d": d_c,
            "r": r_c,
            "wqt4": wqt4, "wkt4": wkt4, "wvt": wvt,
            "biasv": biasv,
        })
    return maps


def kernel(rgb, depth, Wq, bq, Wk, bk, Wv, bv, **run_kwargs):
    nc = _build()
    maps = _in_maps(rgb, depth, Wq, bq, Wk, bk, Wv, bv)
    res = run_bass_kernel_spmd(nc, maps, core_ids=list(range(8)), **run_kwargs)
    results = res.results if hasattr(res, "results") else res
    out = np.empty((B, C, HW), dtype=np.float32)
    for core in range(8):
        b, half = core // 2, core % 2
        out[b][:, half * NQ:(half + 1) * NQ] = results[core]["out"]
    kernel.last_results = res
    return out.reshape(B, C, H, W)
